# revision 36
# baseline (speedup 1.0000x reference)
"""DetailPooling Trainium2 Bass kernel (v5 — engine-balanced f16 pipeline).

Reference computation (per sample, per channel, image [H=256, W=256]):
  eq2   = depthwise 3x3 binomial blur ([1,2,1] (x) [1,2,1] / 16), replicate pad
  eq56  = ((x - eq2)^2 + 1e-12) ** (2*|lam|)
  eq4   = eq56 + |alpha|
  denom = avgpool2x2-stride1(eq4, edge pad bottom/right) + 1e-8
  out   = avgpool2x2-stride2(x * eq4 / denom)

Sharding: pure data parallel, batch 16 -> 8 cores x 2 samples.
Per-core layout: partitions = (b_local, c) = 2*64 = 128, free dim = (h, w),
16 H-tiles of 16 output rows (+3 halo rows), software-pipelined front/back
(front(i+1) issued before back(i) so heads and tails overlap).

Engine split (defaults: EBQ=1 PSD=6 PQR=2 O16=0 REC2=6):
  - DMA: gpsimd SWDGE casts x f32->f16 on load; f32 output via HWDGE (SP).
  - PE:  D16 = 16*x - blur16(x) via shifted-AP accumulating matmuls with
         diagonal f16 stationaries (w-taps -1,-2,-1 on the vertically
         blurred t plus +16 on x; w-replicate edges via 1-column matmuls),
         6 rotating PSUM banks; denominator = 2D 4-tap stride-1 sum of ebq
         into 2-row double-buffered PSUM chunks (bottom row clamped on the
         last tile).
  - ACT: Square((1/16)*D16) straight out of PSUM, Ln(d^2 + 1e-12) in place,
         Exp(2|lam|*ln + ln(0.25)). All funcs live in one activation table;
         redundant LoadActFuncSet instructions are stripped post-compile
         (the stock fixpoint thrashes ~50 reloads = 65 us).
  - DVE: vertical blur pair-adds (s, t), ebq = eb + 0.25|alpha| (= 0.25*eq4,
         shared by numerator AND denominator — pooling the constant yields
         the |alpha| term exactly; the reference's +1e-8 is dropped, ~1e-7
         relative), fnum = x*ebq, reciprocal_approx_fast per PSUM chunk,
         final-pool adds on the parity-split quotient. All f16 packed
         (tensor_tensor 2x mode, tensor_scalar 4x mode).
  - GpSimd: num = fnum * rec (tensor_tensor mult, the only big-ALU form the
         Pool engine supports), written parity-split ("p h w par") so the
         final stride-2 w-pool becomes packed adds.

Notes from walrus (neuronxcc) validation: Pool rejects TensorScalarPtr and
any divide/pow ALU op (tensor_tensor add/mult only); DVE rejects divide;
strided matmul out/moving APs are silently wrong — only dense column
slices are safe; custom DVE ops have no fast perf modes.
Measured: rel err 1.02e-3 (absmax), TimelineSim 297845 ns vs 812910 ns for
the previous all-DVE/ACT fp32 kernel (engine busy: DVE 254, PE 240,
Pool 218, ACT 208; DMA floor ~134).

Scalars (2|lam|, 0.25|alpha|) are specialized at build time from the
runtime lam/alpha values (compile cache keyed on them).
"""

import os
import numpy as np

N_CORES = 8
B, C, H, W = 16, 64, 256, 256
B_LOC = B // N_CORES          # 2 samples per core
P = B_LOC * C                 # 128 partitions
HT = 16                       # output rows (of H) per tile
N_TILES = H // HT             # 16
HO, WO = H // 2, W // 2

_cache = {}

# stage: "a" = DVE highpass, "b" = PE highpass, "c" = +PE pool-vert (default)
STAGE = os.environ.get("KERNEL_STAGE", "c")


def _strip_act_table_loads(nc):
    """All activation funcs used here (square/ln/exp) live together in at
    least one table set; keep a single up-front load of that set and drop
    the rest."""
    import concourse.mybir as mybir
    from concourse.hw_specs import get_activation_tables

    fn = nc.m.functions[0]
    used = set()
    for b in fn.blocks:
        for inst in b.instructions:
            if isinstance(inst, mybir.InstActivation):
                used.add(inst.func)
    if not used:
        return
    tables = list(get_activation_tables(nc.m.arch).items())
    set_id = None
    for i, (name, funcs) in enumerate(tables):
        if used <= funcs:
            set_id = i
            break
    if set_id is None:
        return  # no single table covers everything; leave as-is
    first_done = False
    for b in fn.blocks:
        insts = b.instructions
        kept = []
        changed = False
        for inst in insts:
            if isinstance(inst, mybir.InstLoadActFuncSet):
                if not first_done:
                    inst.act_func_set_id = set_id
                    kept.append(inst)
                    first_done = True
                else:
                    changed = True  # drop
            else:
                kept.append(inst)
        if changed:
            b.instructions[:] = kept


def _build(lam_val=0.6, alpha_val=0.1, stage=None, rep=1):
    import concourse.mybir as mybir
    from concourse import bacc, tile

    stage = stage or STAGE
    f32 = mybir.dt.float32
    f16 = mybir.dt.float16
    i32 = mybir.dt.int32
    Alu = mybir.AluOpType
    Act = mybir.ActivationFunctionType

    la2 = float(2.0 * abs(lam_val))          # exponent scale
    al4 = float(0.25 * abs(alpha_val))       # numerator bias (0.25*|alpha|)
    al8 = float(abs(alpha_val) + 1e-8)       # denominator bias
    lnq = float(np.log(0.25))

    nc = bacc.Bacc("TRN2", target_bir_lowering=False, debug=False,
                   num_devices=N_CORES)
    x_ap = nc.dram_tensor("x", [P, H * W], f32, kind="ExternalInput").ap()
    # lam/alpha still declared so the input map stays uniform (values are
    # baked into the compiled constants; these tensors are unread).
    nc.dram_tensor("lam", [1, 1], f32, kind="ExternalInput")
    nc.dram_tensor("alpha", [1, 1], f32, kind="ExternalInput")
    out_ap = nc.dram_tensor("out", [P, HO * WO], f32, kind="ExternalOutput").ap()

    xd = x_ap.rearrange("p (h w) -> p h w", w=W)      # [128, 256, 256]
    od = out_ap.rearrange("p (h w) -> p h w", w=WO)   # [128, 128, 128]

    use_pe = stage in ("b", "c")
    pe_pool = stage == "c"

    with tile.TileContext(nc) as tc:
        with tc.tile_pool(name="cpool", bufs=1) as cpool, \
             tc.tile_pool(name="pool", bufs=1) as pool, \
             tc.psum_pool(name="pp", bufs=1) as pp:
            eps_t = cpool.tile([P, 1], f32)
            nc.vector.memset(eps_t[:], 1e-12)
            lnq_t = cpool.tile([P, 1], f32)
            nc.vector.memset(lnq_t[:], lnq)
            if use_pe:
                # Diagonal stationaries for the PE taps: iota(j - p) == 0.
                jmp = cpool.tile([P, 128], i32)
                nc.gpsimd.iota(jmp[:], [[1, 128]], base=0, channel_multiplier=-1)
                eye = cpool.tile([P, 128], f16)
                nc.vector.tensor_scalar(eye[:], jmp[:], 0, None, Alu.is_equal)
                dg_m1 = cpool.tile([P, 128], f16)
                nc.vector.tensor_scalar_mul(dg_m1[:], eye[:], -1.0)
                dg_m2 = cpool.tile([P, 128], f16)
                nc.vector.tensor_scalar_mul(dg_m2[:], eye[:], -2.0)
                dg_16 = cpool.tile([P, 128], f16)
                nc.vector.tensor_scalar_mul(dg_16[:], eye[:], 16.0)
                if pe_pool:
                    dg_al8 = cpool.tile([P, 128], f16)
                    nc.vector.tensor_scalar_mul(dg_al8[:], eye[:], al8)
                    ones = cpool.tile([P, 2 * W], f16)
                    nc.vector.memset(ones[:], 1.0)

            def front(i):
                """DMA + blur + d^2 + ln/exp for tile i; returns live tiles."""
                h0 = HT * i
                last = i == N_TILES - 1
                # ---- load x tile as f16 (gpsimd DMA casts f32->f16) ----
                # rows of xb map to image rows h0-1 .. h0+17 (clamped)
                xb = pool.tile([P, HT + 3, W], f16, tag="xb",
                               bufs=int(os.environ.get("KERNEL_XB", "3")))
                xbf = xb[:].rearrange("p h w -> p (h w)")
                if int(os.environ.get("KERNEL_FLATDMA", "0")):
                    # flat 1-D APs so SWDGE descriptors merge per partition
                    if i == 0:
                        nc.gpsimd.dma_start(xbf[:, W:19 * W],
                                            x_ap[:, 0:18 * W])
                        nc.gpsimd.dma_start(xbf[:, 0:W], x_ap[:, 0:W])
                    elif last:
                        nc.gpsimd.dma_start(xbf[:, 0:17 * W],
                                            x_ap[:, (h0 - 1) * W:H * W])
                        nc.gpsimd.dma_start(xbf[:, 17 * W:18 * W],
                                            x_ap[:, (H - 1) * W:H * W])
                        nc.gpsimd.dma_start(xbf[:, 18 * W:19 * W],
                                            x_ap[:, (H - 1) * W:H * W])
                    else:
                        nc.gpsimd.dma_start(
                            xbf[:, :], x_ap[:, (h0 - 1) * W:(h0 + 18) * W])
                elif i == 0:
                    nc.gpsimd.dma_start(xb[:, 1:19, :], xd[:, 0:18, :])
                    nc.gpsimd.dma_start(xb[:, 0:1, :], xd[:, 0:1, :])
                elif last:
                    nc.gpsimd.dma_start(xb[:, 0:17, :], xd[:, h0 - 1:H, :])
                    nc.gpsimd.dma_start(xb[:, 17:18, :], xd[:, H - 1:H, :])
                    nc.gpsimd.dma_start(xb[:, 18:19, :], xd[:, H - 1:H, :])
                else:
                    nc.gpsimd.dma_start(xb[:, :, :], xd[:, h0 - 1:h0 + 18, :])

                # ---- vertical blur: two packed pair-adds ----
                s = pool.tile([P, HT + 2, W], f16, tag="s",
                              bufs=int(os.environ.get("KERNEL_SB", "2")))
                nc.vector.tensor_tensor(s[:], xb[:, 0:18, :], xb[:, 1:19, :],
                                        Alu.add)
                t = pool.tile([P, HT + 1, W], f16, tag="t",
                              bufs=int(os.environ.get("KERNEL_TB", "2")))
                nc.vector.tensor_tensor(t[:], s[:, 0:17, :], s[:, 1:18, :],
                                        Alu.add)

                # ---- d^2 on rows h0..h0+16 ----
                dsq = pool.tile([P, HT + 1, W], f16, tag="dsq",
                                bufs=int(os.environ.get("KERNEL_DSQ", "2")))
                if use_pe:
                    # PE: D16 = 16*x - blur16 via accumulating taps into
                    # PSUM, per <=512-elem chunk (2 rows x 256); 17 rows.
                    # w-replicate edges via 1-column matmuls.
                    for ck in range(9):
                        r0, nr = 2 * ck, (1 if ck == 8 else 2)
                        ps = pp.tile([P, 2 * W], f32, tag="psd",
                                     bufs=(int(os.environ.get("KERNEL_PSD", "6"))
                                           if pe_pool else 8))
                        pc = ps[:, 0:nr * W].rearrange("p (h w) -> p h w", w=W)
                        tr = t[:, r0:r0 + nr, :]
                        nc.tensor.matmul(pc[:, :, 1:W], dg_m1[:],
                                         tr[:, :, 0:W - 1],
                                         start=True, stop=False)
                        nc.tensor.matmul(pc[:, :, 0:1], dg_m1[:],
                                         tr[:, :, 0:1],
                                         start=False, stop=False)
                        nc.tensor.matmul(pc, dg_m2[:], tr,
                                         start=False, stop=False)
                        nc.tensor.matmul(pc[:, :, 0:W - 1], dg_m1[:],
                                         tr[:, :, 1:W],
                                         start=False, stop=False)
                        nc.tensor.matmul(pc[:, :, W - 1:W], dg_m1[:],
                                         tr[:, :, W - 1:W],
                                         start=False, stop=False)
                        nc.tensor.matmul(pc, dg_16[:],
                                         xb[:, 1 + r0:1 + r0 + nr, :],
                                         start=False, stop=True)
                        # ACT evacuates PSUM: dsq = ((1/16)*D16)^2 = d^2
                        nc.scalar.activation(
                            dsq[:, r0:r0 + nr, :], pc, Act.Square,
                            scale=1.0 / 16.0)
                else:
                    # DVE horizontal blur, edges explicit
                    v = pool.tile([P, HT + 1, W], f16, tag="v")
                    nc.vector.tensor_tensor(v[:, :, 0:W - 1], t[:, :, 0:W - 1],
                                            t[:, :, 1:W], Alu.add)
                    nc.vector.tensor_scalar_mul(v[:, :, W - 1:W],
                                                t[:, :, W - 1:W], 2.0)
                    r = pool.tile([P, HT + 1, W], f16, tag="r")
                    nc.vector.tensor_tensor(r[:, :, 1:W], v[:, :, 0:W - 1],
                                            v[:, :, 1:W], Alu.add)
                    # r[0] = v[-1] + v[0] = 2*t[0] + v[0]  (w replicate)
                    nc.vector.scalar_tensor_tensor(
                        r[:, :, 0:1], t[:, :, 0:1], 2.0, v[:, :, 0:1],
                        Alu.mult, Alu.add)
                    x16 = pool.tile([P, HT + 1, W], f16, tag="x16")
                    nc.vector.tensor_scalar_mul(x16[:], xb[:, 1:18, :], 16.0)
                    d16 = pool.tile([P, HT + 1, W], f16, tag="d16")
                    nc.vector.tensor_tensor(d16[:], x16[:], r[:], Alu.subtract)
                    # dsq = (D16)^2; the 1/256 rides the Ln scale below
                    nc.vector.tensor_tensor(dsq[:], d16[:], d16[:], Alu.mult)

                # ---- pow via ln/exp:  eb = 0.25 * (d^2 + 1e-12)^(2|lam|) ----
                # Ln in place on dsq (f16 log is plenty for the 2e-2 gate)
                nc.scalar.activation(dsq[:], dsq[:], Act.Ln, bias=eps_t[:],
                                     scale=1.0 if use_pe else 1.0 / 256.0)
                ebx = pool.tile([P, HT + 1, W + 1], f16, tag="ebx",
                                bufs=int(os.environ.get("KERNEL_EBX", "2")))
                nc.scalar.activation(ebx[:, :, 0:W], dsq[:], Act.Exp,
                                     scale=la2, bias=lnq_t[:])
                # replicate-pad right edge for the stride-1 pool
                nc.vector.tensor_copy(ebx[:, :, W:W + 1], ebx[:, :, W - 1:W])
                return xb, ebx

            def back(i, xb, ebx):
                """pools + divide + final pool + store for tile i."""
                last = i == N_TILES - 1
                ebq_mode = pe_pool and int(os.environ.get("KERNEL_EBQ", "1"))
                fnum = pool.tile([P, HT, W], f16, tag="fnum",
                                 bufs=int(os.environ.get("KERNEL_FB", "2")))
                if ebq_mode:
                    # ebq = eb + 0.25|alpha| = 0.25*eq4 serves BOTH paths:
                    # fnum = x*ebq, and den = 2x2 stride-1 sum of ebq (the
                    # pooled constant contributes the |alpha| term exactly;
                    # the reference's extra 1e-8 is dropped, ~1e-7 relative).
                    ebq = pool.tile([P, HT + 1, W + 1], f16, tag="ebq",
                                    bufs=2)
                    nc.vector.tensor_scalar_add(ebq[:, :, 0:W],
                                                ebx[:, :, 0:W], al4)
                    nc.vector.tensor_copy(ebq[:, :, W:W + 1],
                                          ebq[:, :, W - 1:W])
                    nc.vector.tensor_tensor(fnum[:], xb[:, 1:17, :],
                                            ebq[:, 0:HT, 0:W], Alu.mult)
                else:
                    eq4q = pool.tile([P, HT, W], f16, tag="eq4q", bufs=2)
                    nc.vector.tensor_scalar_add(eq4q[:], ebx[:, 0:HT, 0:W],
                                                al4)
                    nc.vector.tensor_tensor(fnum[:], xb[:, 1:17, :], eq4q[:],
                                            Alu.mult)

                # ---- denominator pool ----
                p1pe = pe_pool and int(os.environ.get("KERNEL_P1PE", "0"))
                if not p1pe and not ebq_mode:
                    p1 = pool.tile([P, HT + 1, W], f16, tag="p1", bufs=2)
                    nc.vector.tensor_tensor(p1[:], ebx[:, :, 0:W],
                                            ebx[:, :, 1:W + 1], Alu.add)
                # eq7 layout: [P][par(2)][h(16)][w2(128)], flat
                eq7 = pool.tile([P, 2 * HT * WO], f16, tag="eq7",
                                bufs=int(os.environ.get("KERNEL_E7", "2")))
                eq7v = eq7[:].rearrange("p (par h w) -> p h w par",
                                        par=2, h=HT)
                if pe_pool and ebq_mode and int(os.environ.get("KERNEL_PQR", "2")) == 2:
                    # 2-row pool chunks: one matmul per tap, psq double-
                    # buffered so PE-pool(ck+1) overlaps DVE recip(ck).
                    for ck in range(8):
                        r0 = 2 * ck
                        psq = pp.tile([P, 2 * W], f32, tag="psq",
                                      bufs=int(os.environ.get("KERNEL_PSQ2", "2")))
                        pq = psq[:].rearrange("p (h w) -> p h w", w=W)
                        bottom = last and ck == 7
                        nc.tensor.matmul(pq, eye[:], ebq[:, r0:r0 + 2, 0:W],
                                         start=True, stop=False)
                        nc.tensor.matmul(pq, eye[:], ebq[:, r0:r0 + 2, 1:W + 1],
                                         start=False, stop=False)
                        if bottom:
                            for pi, pr in enumerate((0, 1)):
                                st = pi == 1
                                nc.tensor.matmul(pq[:, pr:pr + 1, :], eye[:],
                                                 ebq[:, 15:16, 0:W],
                                                 start=False, stop=False)
                                nc.tensor.matmul(pq[:, pr:pr + 1, :], eye[:],
                                                 ebq[:, 15:16, 1:W + 1],
                                                 start=False, stop=st)
                        else:
                            nc.tensor.matmul(pq, eye[:],
                                             ebq[:, r0 + 1:r0 + 3, 0:W],
                                             start=False, stop=False)
                            nc.tensor.matmul(pq, eye[:],
                                             ebq[:, r0 + 1:r0 + 3, 1:W + 1],
                                             start=False, stop=True)
                        rec = pool.tile([P, 2 * W], f32, tag="rec2",
                                        bufs=int(os.environ.get("KERNEL_REC2", "6")))
                        nc.vector.reciprocal_approx_fast(rec[:], psq[:])
                        fn_c = fnum[:, r0:r0 + 2, :].rearrange(
                            "p h (w par) -> p h w par", par=2)
                        rc_c = rec[:].rearrange(
                            "p (h w par) -> p h w par", par=2, w=WO)
                        nc.gpsimd.tensor_tensor(
                            eq7v[:, r0:r0 + 2, :, :], fn_c, rc_c, Alu.mult)
                elif pe_pool:
                    # PE: den = 2x2 stride-1 sum of eb + al8*ones into PSUM
                    # (4-row chunks; 2 matmuls per tap due to the 512 moving
                    # limit). With KERNEL_P1PE the four 2D taps read ebx
                    # directly (its pad column handles the w edge); else two
                    # vertical taps on the DVE-computed p1.
                    # DVE reciprocal straight from PSUM; GPS multiplies.
                    for ck in range(4):
                        r0 = 4 * ck
                        psq = pp.tile([P, 4 * W], f32, tag="psq",
                                      bufs=int(os.environ.get("KERNEL_PSQ", "1")))
                        pq = psq[:].rearrange("p (h w) -> p h w", w=W)
                        onv = ones[:].rearrange("p (h w) -> p h w", w=W)
                        for sub in (0, 2):
                            rs = r0 + sub
                            pqs = pq[:, sub:sub + 2, :]
                            bottom = last and ck == 3 and sub == 2
                            if ebq_mode:
                                # 2D 4-tap pool of ebq; no bias matmul
                                nc.tensor.matmul(pqs, eye[:],
                                                 ebq[:, rs:rs + 2, 0:W],
                                                 start=True, stop=False)
                                nc.tensor.matmul(pqs, eye[:],
                                                 ebq[:, rs:rs + 2, 1:W + 1],
                                                 start=False, stop=False)
                                if bottom:
                                    # rows 14,15: the h+1 taps clamp to 15
                                    for pi, pr in enumerate((2, 3)):
                                        st = pi == 1
                                        nc.tensor.matmul(
                                            pq[:, pr:pr + 1, :], eye[:],
                                            ebq[:, 15:16, 0:W],
                                            start=False, stop=False)
                                        nc.tensor.matmul(
                                            pq[:, pr:pr + 1, :], eye[:],
                                            ebq[:, 15:16, 1:W + 1],
                                            start=False, stop=st)
                                else:
                                    nc.tensor.matmul(pqs, eye[:],
                                                     ebq[:, rs + 1:rs + 3,
                                                         0:W],
                                                     start=False, stop=False)
                                    nc.tensor.matmul(pqs, eye[:],
                                                     ebq[:, rs + 1:rs + 3,
                                                         1:W + 1],
                                                     start=False, stop=True)
                                continue
                            if p1pe:
                                nc.tensor.matmul(pqs, eye[:],
                                                 ebx[:, rs:rs + 2, 0:W],
                                                 start=True, stop=False)
                                nc.tensor.matmul(pqs, eye[:],
                                                 ebx[:, rs:rs + 2, 1:W + 1],
                                                 start=False, stop=False)
                                if bottom:
                                    # rows 14,15: the h+1 taps clamp to 15
                                    for pr in (2, 3):
                                        nc.tensor.matmul(
                                            pq[:, pr:pr + 1, :], eye[:],
                                            ebx[:, 15:16, 0:W],
                                            start=False, stop=False)
                                        nc.tensor.matmul(
                                            pq[:, pr:pr + 1, :], eye[:],
                                            ebx[:, 15:16, 1:W + 1],
                                            start=False, stop=False)
                                else:
                                    nc.tensor.matmul(pqs, eye[:],
                                                     ebx[:, rs + 1:rs + 3, 0:W],
                                                     start=False, stop=False)
                                    nc.tensor.matmul(pqs, eye[:],
                                                     ebx[:, rs + 1:rs + 3,
                                                         1:W + 1],
                                                     start=False, stop=False)
                            else:
                                nc.tensor.matmul(pqs, eye[:],
                                                 p1[:, rs:rs + 2, :],
                                                 start=True, stop=False)
                                if bottom:
                                    # bottom image row: pv[15]=2*p1[15]+al8
                                    nc.tensor.matmul(pq[:, 2:3, :], eye[:],
                                                     p1[:, rs + 1:rs + 2, :],
                                                     start=False, stop=False)
                                    nc.tensor.matmul(pq[:, 3:4, :], eye[:],
                                                     p1[:, rs + 1:rs + 2, :],
                                                     start=False, stop=False)
                                else:
                                    nc.tensor.matmul(pqs, eye[:],
                                                     p1[:, rs + 1:rs + 3, :],
                                                     start=False, stop=False)
                            nc.tensor.matmul(pqs, dg_al8[:], onv,
                                             start=False, stop=True)
                        rec = pool.tile([P, 4 * W], f32, tag="rec",
                                        bufs=int(os.environ.get("KERNEL_REC", "4")))
                        nc.vector.reciprocal_approx_fast(rec[:], psq[:])
                        fn_c = fnum[:, r0:r0 + 4, :].rearrange(
                            "p h (w par) -> p h w par", par=2)
                        rc_c = rec[:].rearrange(
                            "p (h w par) -> p h w par", par=2, w=WO)
                        nc.gpsimd.tensor_tensor(
                            eq7v[:, r0:r0 + 4, :, :], fn_c, rc_c, Alu.mult)
                else:
                    pv = pool.tile([P, HT, W], f16, tag="pv")
                    if last:
                        nc.vector.tensor_tensor(
                            pv[:, 0:15, :], p1[:, 0:15, :], p1[:, 1:16, :],
                            Alu.add)
                        nc.vector.tensor_scalar_mul(
                            pv[:, 15:16, :], p1[:, 15:16, :], 2.0)
                    else:
                        nc.vector.tensor_tensor(pv[:], p1[:, 0:16, :],
                                                p1[:, 1:17, :], Alu.add)
                    den = pool.tile([P, HT, W], f32, tag="den")
                    nc.vector.tensor_scalar_add(den[:], pv[:], al8)
                    rec = pool.tile([P, HT, W], f32, tag="recf")
                    nc.vector.reciprocal_approx_fast(
                        rec[:].rearrange("p h w -> p (h w)"),
                        den[:].rearrange("p h w -> p (h w)"))
                    fn_c = fnum[:].rearrange("p h (w par) -> p h w par", par=2)
                    rc_c = rec[:].rearrange("p h (w par) -> p h w par", par=2)
                    nc.gpsimd.tensor_tensor(eq7v, fn_c, rc_c, Alu.mult)

                # ---- final pool: packed adds on the parity halves ----
                e0 = eq7[:, 0:HT * WO].rearrange("p (h w) -> p h w", w=WO)
                e1 = eq7[:, HT * WO:2 * HT * WO].rearrange(
                    "p (h w) -> p h w", w=WO)
                if pe_pool and int(os.environ.get("KERNEL_QPE", "0")):
                    o_t = pool.tile([P, HT // 2, WO], f32, tag="o", bufs=2)
                    # PE: o = e0[2j] + e0[2j+1] + e1[2j] + e1[2j+1] via 4
                    # accumulating taps (2 512-elem chunks), DVE evacuates.
                    po = pp.tile([P, (HT // 2) * WO], f32, tag="po",
                                 bufs=int(os.environ.get("KERNEL_PO", "2")))
                    pov = po[:].rearrange("p (h w) -> p h w", w=WO)
                    for jh in (0, 1):
                        pc = pov[:, 4 * jh:4 * jh + 4, :]
                        r0 = 8 * jh
                        nc.tensor.matmul(pc, eye[:], e0[:, r0:r0 + 8:2, :],
                                         start=True, stop=False)
                        nc.tensor.matmul(pc, eye[:], e0[:, r0 + 1:r0 + 8:2, :],
                                         start=False, stop=False)
                        nc.tensor.matmul(pc, eye[:], e1[:, r0:r0 + 8:2, :],
                                         start=False, stop=False)
                        nc.tensor.matmul(pc, eye[:], e1[:, r0 + 1:r0 + 8:2, :],
                                         start=False, stop=True)
                    nc.vector.tensor_copy(
                        o_t[:].rearrange("p h w -> p (h w)"), po[:])
                elif int(os.environ.get("KERNEL_O16", "0")):
                    q = pool.tile([P, HT, WO], f16, tag="q", bufs=2)
                    nc.vector.tensor_tensor(q[:], e0, e1, Alu.add)
                    o16 = pool.tile([P, HT // 2, WO], f16, tag="o16", bufs=2)
                    nc.vector.tensor_tensor(o16[:], q[:, 0:HT:2, :],
                                            q[:, 1:HT:2, :], Alu.add)
                    # gpsimd DMA casts f16 -> f32 on store
                    nc.gpsimd.dma_start(
                        od[:, (HT // 2) * i:(HT // 2) * (i + 1), :], o16[:])
                    return
                else:
                    q = pool.tile([P, HT, WO], f16, tag="q", bufs=2)
                    nc.vector.tensor_tensor(q[:], e0, e1, Alu.add)
                    o_t = pool.tile([P, HT // 2, WO], f32, tag="o", bufs=2)
                    nc.vector.tensor_tensor(o_t[:], q[:, 0:HT:2, :],
                                            q[:, 1:HT:2, :], Alu.add)
                nc.sync.dma_start(od[:, (HT // 2) * i:(HT // 2) * (i + 1), :],
                                  o_t[:])

            # software pipeline: front(i+1) is issued before back(i) so the
            # scheduler can overlap the PE/ACT head of one tile with the
            # DVE/GpSimd tail of the previous.
            LAG = int(os.environ.get("KERNEL_LAG", "1"))
            n_flat = rep * N_TILES
            live = {}
            for ii in range(n_flat + LAG):
                if ii < n_flat:
                    live[ii] = front(ii % N_TILES)
                jj = ii - LAG
                if jj >= 0:
                    xb_j, ebx_j = live.pop(jj)
                    back(jj % N_TILES, xb_j, ebx_j)
    nc.compile()
    _strip_act_table_loads(nc)
    return nc


def _get_nc(lam_val, alpha_val):
    key = ("nc", float(lam_val), float(alpha_val), STAGE)
    if key not in _cache:
        _cache[key] = _build(lam_val, alpha_val)
    return _cache[key]


def kernel(x, lam, alpha):
    if not int(os.environ.get("KERNEL_TRACE", "0")):
        os.environ["BASS_NEVER_TRACE"] = "1"
    # The harness may pin JAX_PLATFORMS=cpu for its jax reference; that would
    # mask the axon NeuronCore devices this kernel dispatches to. Clear it
    # before jax's backend initializes (no-op if jax already initialized).
    jp = os.environ.get("JAX_PLATFORMS")
    if jp and "axon" not in jp:
        del os.environ["JAX_PLATFORMS"]
    import concourse.bass_utils as bass_utils

    x = np.ascontiguousarray(np.asarray(x, dtype=np.float32))
    lam = np.asarray(lam, dtype=np.float32).reshape(1, 1)
    alpha = np.asarray(alpha, dtype=np.float32).reshape(1, 1)
    assert x.shape == (B, C, H, W)

    nc = _get_nc(float(lam[0, 0]), float(alpha[0, 0]))
    in_maps = []
    for i in range(N_CORES):
        shard = x[i * B_LOC:(i + 1) * B_LOC].reshape(P, H * W)
        in_maps.append({"x": np.ascontiguousarray(shard),
                        "lam": lam, "alpha": alpha})

    res = bass_utils.run_bass_kernel_spmd(
        nc, in_maps, core_ids=list(range(N_CORES)),
        trace=bool(int(os.environ.get("KERNEL_TRACE", "0"))))
    _cache["last_results"] = res

    out = np.empty((B, C, HO, WO), dtype=np.float32)
    for i in range(N_CORES):
        out[i * B_LOC:(i + 1) * B_LOC] = \
            res.results[i]["out"].reshape(B_LOC, C, HO, WO)
    return out


# revision 37
# speedup vs baseline: 1.0099x; 1.0099x over previous
"""DetailPooling Trainium2 Bass kernel (v5 — engine-balanced f16 pipeline).

Reference computation (per sample, per channel, image [H=256, W=256]):
  eq2   = depthwise 3x3 binomial blur ([1,2,1] (x) [1,2,1] / 16), replicate pad
  eq56  = ((x - eq2)^2 + 1e-12) ** (2*|lam|)
  eq4   = eq56 + |alpha|
  denom = avgpool2x2-stride1(eq4, edge pad bottom/right) + 1e-8
  out   = avgpool2x2-stride2(x * eq4 / denom)

Sharding: pure data parallel, batch 16 -> 8 cores x 2 samples.
Per-core layout: partitions = (b_local, c) = 2*64 = 128, free dim = (h, w),
16 H-tiles of 16 output rows (+3 halo rows), software-pipelined front/back
(front(i+1) issued before back(i) so heads and tails overlap).

Engine split (defaults: EBQ=1 PSD=6 PQR=2 O16=0 REC2=6):
  - DMA: gpsimd SWDGE casts x f32->f16 on load; f32 output via HWDGE (SP).
  - PE:  D16 = 16*x - blur16(x) via shifted-AP accumulating matmuls with
         diagonal f16 stationaries (w-taps -1,-2,-1 on the vertically
         blurred t plus +16 on x; w-replicate edges via 1-column matmuls),
         6 rotating PSUM banks; denominator = 2D 4-tap stride-1 sum of ebq
         into 2-row double-buffered PSUM chunks (bottom row clamped on the
         last tile).
  - ACT: Square((1/16)*D16) straight out of PSUM, Ln(d^2 + 1e-12) in place,
         Exp(2|lam|*ln + ln(0.25)). All funcs live in one activation table;
         redundant LoadActFuncSet instructions are stripped post-compile
         (the stock fixpoint thrashes ~50 reloads = 65 us).
  - DVE: vertical blur pair-adds (s, t), ebq = eb + 0.25|alpha| (= 0.25*eq4,
         shared by numerator AND denominator — pooling the constant yields
         the |alpha| term exactly; the reference's +1e-8 is dropped, ~1e-7
         relative), fnum = x*ebq, reciprocal_approx_fast per PSUM chunk,
         final-pool adds on the parity-split quotient. All f16 packed
         (tensor_tensor 2x mode, tensor_scalar 4x mode).
  - GpSimd: num = fnum * rec (tensor_tensor mult, the only big-ALU form the
         Pool engine supports), written parity-split ("p h w par") so the
         final stride-2 w-pool becomes packed adds.

Notes from walrus (neuronxcc) validation: Pool rejects TensorScalarPtr and
any divide/pow ALU op (tensor_tensor add/mult only); DVE rejects divide;
strided matmul out/moving APs are silently wrong — only dense column
slices are safe; custom DVE ops have no fast perf modes.
Measured: rel err 1.02e-3 (absmax), TimelineSim 297845 ns vs 812910 ns for
the previous all-DVE/ACT fp32 kernel (engine busy: DVE 254, PE 240,
Pool 218, ACT 208; DMA floor ~134).

Scalars (2|lam|, 0.25|alpha|) are specialized at build time from the
runtime lam/alpha values (compile cache keyed on them).
"""

import os
import numpy as np

N_CORES = 8
B, C, H, W = 16, 64, 256, 256
B_LOC = B // N_CORES          # 2 samples per core
P = B_LOC * C                 # 128 partitions
HT = 16                       # output rows (of H) per tile
N_TILES = H // HT             # 16
HO, WO = H // 2, W // 2

_cache = {}

# stage: "a" = DVE highpass, "b" = PE highpass, "c" = +PE pool-vert (default)
STAGE = os.environ.get("KERNEL_STAGE", "c")


def _strip_act_table_loads(nc):
    """All activation funcs used here (square/ln/exp) live together in at
    least one table set; keep a single up-front load of that set and drop
    the rest."""
    import concourse.mybir as mybir
    from concourse.hw_specs import get_activation_tables

    fn = nc.m.functions[0]
    used = set()
    for b in fn.blocks:
        for inst in b.instructions:
            if isinstance(inst, mybir.InstActivation):
                used.add(inst.func)
    if not used:
        return
    tables = list(get_activation_tables(nc.m.arch).items())
    set_id = None
    for i, (name, funcs) in enumerate(tables):
        if used <= funcs:
            set_id = i
            break
    if set_id is None:
        return  # no single table covers everything; leave as-is
    first_done = False
    for b in fn.blocks:
        insts = b.instructions
        kept = []
        changed = False
        for inst in insts:
            if isinstance(inst, mybir.InstLoadActFuncSet):
                if not first_done:
                    inst.act_func_set_id = set_id
                    kept.append(inst)
                    first_done = True
                else:
                    changed = True  # drop
            else:
                kept.append(inst)
        if changed:
            b.instructions[:] = kept


def _build(lam_val=0.6, alpha_val=0.1, stage=None, rep=1):
    import concourse.mybir as mybir
    from concourse import bacc, tile

    stage = stage or STAGE
    f32 = mybir.dt.float32
    f16 = mybir.dt.float16
    i32 = mybir.dt.int32
    Alu = mybir.AluOpType
    Act = mybir.ActivationFunctionType

    la2 = float(2.0 * abs(lam_val))          # exponent scale
    al4 = float(0.25 * abs(alpha_val))       # numerator bias (0.25*|alpha|)
    al8 = float(abs(alpha_val) + 1e-8)       # denominator bias
    lnq = float(np.log(0.25))

    nc = bacc.Bacc("TRN2", target_bir_lowering=False, debug=False,
                   num_devices=N_CORES)
    x_ap = nc.dram_tensor("x", [P, H * W], f32, kind="ExternalInput").ap()
    # lam/alpha still declared so the input map stays uniform (values are
    # baked into the compiled constants; these tensors are unread).
    nc.dram_tensor("lam", [1, 1], f32, kind="ExternalInput")
    nc.dram_tensor("alpha", [1, 1], f32, kind="ExternalInput")
    out_ap = nc.dram_tensor("out", [P, HO * WO], f32, kind="ExternalOutput").ap()

    xd = x_ap.rearrange("p (h w) -> p h w", w=W)      # [128, 256, 256]
    od = out_ap.rearrange("p (h w) -> p h w", w=WO)   # [128, 128, 128]

    use_pe = stage in ("b", "c")
    pe_pool = stage == "c"

    with tile.TileContext(nc) as tc:
        with tc.tile_pool(name="cpool", bufs=1) as cpool, \
             tc.tile_pool(name="pool", bufs=1) as pool, \
             tc.psum_pool(name="pp", bufs=1) as pp:
            eps_t = cpool.tile([P, 1], f32)
            nc.vector.memset(eps_t[:], 1e-12)
            lnq_t = cpool.tile([P, 1], f32)
            nc.vector.memset(lnq_t[:], lnq)
            if use_pe:
                # Diagonal stationaries for the PE taps: iota(j - p) == 0.
                jmp = cpool.tile([P, 128], i32)
                nc.gpsimd.iota(jmp[:], [[1, 128]], base=0, channel_multiplier=-1)
                eye = cpool.tile([P, 128], f16)
                nc.vector.tensor_scalar(eye[:], jmp[:], 0, None, Alu.is_equal)
                dg_m1 = cpool.tile([P, 128], f16)
                nc.vector.tensor_scalar_mul(dg_m1[:], eye[:], -1.0)
                dg_m2 = cpool.tile([P, 128], f16)
                nc.vector.tensor_scalar_mul(dg_m2[:], eye[:], -2.0)
                dg_16 = cpool.tile([P, 128], f16)
                nc.vector.tensor_scalar_mul(dg_16[:], eye[:], 16.0)
                if pe_pool:
                    dg_al8 = cpool.tile([P, 128], f16)
                    nc.vector.tensor_scalar_mul(dg_al8[:], eye[:], al8)
                    ones = cpool.tile([P, 2 * W], f16)
                    nc.vector.memset(ones[:], 1.0)

            def front(i):
                """DMA + blur + d^2 + ln/exp for tile i; returns live tiles."""
                h0 = HT * i
                last = i == N_TILES - 1
                # ---- load x tile as f16 (gpsimd DMA casts f32->f16) ----
                # rows of xb map to image rows h0-1 .. h0+17 (clamped)
                xb = pool.tile([P, HT + 3, W], f16, tag="xb",
                               bufs=int(os.environ.get("KERNEL_XB", "3")))
                xbf = xb[:].rearrange("p h w -> p (h w)")
                if int(os.environ.get("KERNEL_FLATDMA", "0")):
                    # flat 1-D APs so SWDGE descriptors merge per partition
                    if i == 0:
                        nc.gpsimd.dma_start(xbf[:, W:19 * W],
                                            x_ap[:, 0:18 * W])
                        nc.gpsimd.dma_start(xbf[:, 0:W], x_ap[:, 0:W])
                    elif last:
                        nc.gpsimd.dma_start(xbf[:, 0:17 * W],
                                            x_ap[:, (h0 - 1) * W:H * W])
                        nc.gpsimd.dma_start(xbf[:, 17 * W:18 * W],
                                            x_ap[:, (H - 1) * W:H * W])
                        nc.gpsimd.dma_start(xbf[:, 18 * W:19 * W],
                                            x_ap[:, (H - 1) * W:H * W])
                    else:
                        nc.gpsimd.dma_start(
                            xbf[:, :], x_ap[:, (h0 - 1) * W:(h0 + 18) * W])
                elif i == 0:
                    nc.gpsimd.dma_start(xb[:, 1:19, :], xd[:, 0:18, :])
                    nc.gpsimd.dma_start(xb[:, 0:1, :], xd[:, 0:1, :])
                elif last:
                    nc.gpsimd.dma_start(xb[:, 0:17, :], xd[:, h0 - 1:H, :])
                    nc.gpsimd.dma_start(xb[:, 17:18, :], xd[:, H - 1:H, :])
                    nc.gpsimd.dma_start(xb[:, 18:19, :], xd[:, H - 1:H, :])
                else:
                    nc.gpsimd.dma_start(xb[:, :, :], xd[:, h0 - 1:h0 + 18, :])

                # ---- vertical blur: two packed pair-adds ----
                s = pool.tile([P, HT + 2, W], f16, tag="s",
                              bufs=int(os.environ.get("KERNEL_SB", "2")))
                nc.vector.tensor_tensor(s[:], xb[:, 0:18, :], xb[:, 1:19, :],
                                        Alu.add)
                t = pool.tile([P, HT + 1, W], f16, tag="t",
                              bufs=int(os.environ.get("KERNEL_TB", "2")))
                nc.vector.tensor_tensor(t[:], s[:, 0:17, :], s[:, 1:18, :],
                                        Alu.add)

                # ---- d^2 on rows h0..h0+16 ----
                dsq = pool.tile([P, HT + 1, W], f16, tag="dsq",
                                bufs=int(os.environ.get("KERNEL_DSQ", "2")))
                if use_pe:
                    # PE: D16 = 16*x - blur16 via accumulating taps into
                    # PSUM, per <=512-elem chunk (2 rows x 256); 17 rows.
                    # w-replicate edges via 1-column matmuls.
                    for ck in range(9):
                        r0, nr = 2 * ck, (1 if ck == 8 else 2)
                        ps = pp.tile([P, 2 * W], f32, tag="psd",
                                     bufs=(int(os.environ.get("KERNEL_PSD", "6"))
                                           if pe_pool else 8))
                        pc = ps[:, 0:nr * W].rearrange("p (h w) -> p h w", w=W)
                        tr = t[:, r0:r0 + nr, :]
                        nc.tensor.matmul(pc[:, :, 1:W], dg_m1[:],
                                         tr[:, :, 0:W - 1],
                                         start=True, stop=False)
                        nc.tensor.matmul(pc[:, :, 0:1], dg_m1[:],
                                         tr[:, :, 0:1],
                                         start=False, stop=False)
                        nc.tensor.matmul(pc, dg_m2[:], tr,
                                         start=False, stop=False)
                        nc.tensor.matmul(pc[:, :, 0:W - 1], dg_m1[:],
                                         tr[:, :, 1:W],
                                         start=False, stop=False)
                        nc.tensor.matmul(pc[:, :, W - 1:W], dg_m1[:],
                                         tr[:, :, W - 1:W],
                                         start=False, stop=False)
                        nc.tensor.matmul(pc, dg_16[:],
                                         xb[:, 1 + r0:1 + r0 + nr, :],
                                         start=False, stop=True)
                        # ACT evacuates PSUM: dsq = ((1/16)*D16)^2 = d^2
                        nc.scalar.activation(
                            dsq[:, r0:r0 + nr, :], pc, Act.Square,
                            scale=1.0 / 16.0)
                else:
                    # DVE horizontal blur, edges explicit
                    v = pool.tile([P, HT + 1, W], f16, tag="v")
                    nc.vector.tensor_tensor(v[:, :, 0:W - 1], t[:, :, 0:W - 1],
                                            t[:, :, 1:W], Alu.add)
                    nc.vector.tensor_scalar_mul(v[:, :, W - 1:W],
                                                t[:, :, W - 1:W], 2.0)
                    r = pool.tile([P, HT + 1, W], f16, tag="r")
                    nc.vector.tensor_tensor(r[:, :, 1:W], v[:, :, 0:W - 1],
                                            v[:, :, 1:W], Alu.add)
                    # r[0] = v[-1] + v[0] = 2*t[0] + v[0]  (w replicate)
                    nc.vector.scalar_tensor_tensor(
                        r[:, :, 0:1], t[:, :, 0:1], 2.0, v[:, :, 0:1],
                        Alu.mult, Alu.add)
                    x16 = pool.tile([P, HT + 1, W], f16, tag="x16")
                    nc.vector.tensor_scalar_mul(x16[:], xb[:, 1:18, :], 16.0)
                    d16 = pool.tile([P, HT + 1, W], f16, tag="d16")
                    nc.vector.tensor_tensor(d16[:], x16[:], r[:], Alu.subtract)
                    # dsq = (D16)^2; the 1/256 rides the Ln scale below
                    nc.vector.tensor_tensor(dsq[:], d16[:], d16[:], Alu.mult)

                # ---- pow via ln/exp:  eb = 0.25 * (d^2 + 1e-12)^(2|lam|) ----
                # Ln in place on dsq (f16 log is plenty for the 2e-2 gate)
                nc.scalar.activation(dsq[:], dsq[:], Act.Ln, bias=eps_t[:],
                                     scale=1.0 if use_pe else 1.0 / 256.0)
                ebx = pool.tile([P, HT + 1, W + 1], f16, tag="ebx",
                                bufs=int(os.environ.get("KERNEL_EBX", "2")))
                nc.scalar.activation(ebx[:, :, 0:W], dsq[:], Act.Exp,
                                     scale=la2, bias=lnq_t[:])
                # replicate-pad right edge for the stride-1 pool
                nc.vector.tensor_copy(ebx[:, :, W:W + 1], ebx[:, :, W - 1:W])
                return xb, ebx

            def back(i, xb, ebx):
                """pools + divide + final pool + store for tile i."""
                last = i == N_TILES - 1
                ebq_mode = pe_pool and int(os.environ.get("KERNEL_EBQ", "1"))
                fnum = pool.tile([P, HT, W], f16, tag="fnum",
                                 bufs=int(os.environ.get("KERNEL_FB", "2")))
                if ebq_mode:
                    # ebq = eb + 0.25|alpha| = 0.25*eq4 serves BOTH paths:
                    # fnum = x*ebq, and den = 2x2 stride-1 sum of ebq (the
                    # pooled constant contributes the |alpha| term exactly;
                    # the reference's extra 1e-8 is dropped, ~1e-7 relative).
                    ebq = pool.tile([P, HT + 1, W + 1], f16, tag="ebq",
                                    bufs=2)
                    nc.vector.tensor_scalar_add(ebq[:, :, 0:W],
                                                ebx[:, :, 0:W], al4)
                    nc.vector.tensor_copy(ebq[:, :, W:W + 1],
                                          ebq[:, :, W - 1:W])
                    nc.vector.tensor_tensor(fnum[:], xb[:, 1:17, :],
                                            ebq[:, 0:HT, 0:W], Alu.mult)
                else:
                    eq4q = pool.tile([P, HT, W], f16, tag="eq4q", bufs=2)
                    nc.vector.tensor_scalar_add(eq4q[:], ebx[:, 0:HT, 0:W],
                                                al4)
                    nc.vector.tensor_tensor(fnum[:], xb[:, 1:17, :], eq4q[:],
                                            Alu.mult)

                # ---- denominator pool ----
                p1pe = pe_pool and int(os.environ.get("KERNEL_P1PE", "0"))
                if not p1pe and not ebq_mode:
                    p1 = pool.tile([P, HT + 1, W], f16, tag="p1", bufs=2)
                    nc.vector.tensor_tensor(p1[:], ebx[:, :, 0:W],
                                            ebx[:, :, 1:W + 1], Alu.add)
                # eq7 layout: [P][par(2)][h(16)][w2(128)], flat
                eq7 = pool.tile([P, 2 * HT * WO], f16, tag="eq7",
                                bufs=int(os.environ.get("KERNEL_E7", "2")))
                eq7v = eq7[:].rearrange("p (par h w) -> p h w par",
                                        par=2, h=HT)
                if pe_pool and ebq_mode and int(os.environ.get("KERNEL_PQR", "2")) == 2:
                    # 2-row pool chunks: one matmul per tap, psq double-
                    # buffered so PE-pool(ck+1) overlaps DVE recip(ck).
                    for ck in range(8):
                        r0 = 2 * ck
                        psq = pp.tile([P, 2 * W], f32, tag="psq",
                                      bufs=int(os.environ.get("KERNEL_PSQ2", "2")))
                        pq = psq[:].rearrange("p (h w) -> p h w", w=W)
                        bottom = last and ck == 7
                        nc.tensor.matmul(pq, eye[:], ebq[:, r0:r0 + 2, 0:W],
                                         start=True, stop=False)
                        nc.tensor.matmul(pq, eye[:], ebq[:, r0:r0 + 2, 1:W + 1],
                                         start=False, stop=False)
                        if bottom:
                            for pi, pr in enumerate((0, 1)):
                                st = pi == 1
                                nc.tensor.matmul(pq[:, pr:pr + 1, :], eye[:],
                                                 ebq[:, 15:16, 0:W],
                                                 start=False, stop=False)
                                nc.tensor.matmul(pq[:, pr:pr + 1, :], eye[:],
                                                 ebq[:, 15:16, 1:W + 1],
                                                 start=False, stop=st)
                        else:
                            nc.tensor.matmul(pq, eye[:],
                                             ebq[:, r0 + 1:r0 + 3, 0:W],
                                             start=False, stop=False)
                            nc.tensor.matmul(pq, eye[:],
                                             ebq[:, r0 + 1:r0 + 3, 1:W + 1],
                                             start=False, stop=True)
                        rec = pool.tile([P, 2 * W], f32, tag="rec2",
                                        bufs=int(os.environ.get("KERNEL_REC2", "6")))
                        nc.vector.reciprocal_approx_fast(rec[:], psq[:])
                        fn_c = fnum[:, r0:r0 + 2, :].rearrange(
                            "p h (w par) -> p h w par", par=2)
                        rc_c = rec[:].rearrange(
                            "p (h w par) -> p h w par", par=2, w=WO)
                        nc.gpsimd.tensor_tensor(
                            eq7v[:, r0:r0 + 2, :, :], fn_c, rc_c, Alu.mult)
                elif pe_pool:
                    # PE: den = 2x2 stride-1 sum of eb + al8*ones into PSUM
                    # (4-row chunks; 2 matmuls per tap due to the 512 moving
                    # limit). With KERNEL_P1PE the four 2D taps read ebx
                    # directly (its pad column handles the w edge); else two
                    # vertical taps on the DVE-computed p1.
                    # DVE reciprocal straight from PSUM; GPS multiplies.
                    for ck in range(4):
                        r0 = 4 * ck
                        psq = pp.tile([P, 4 * W], f32, tag="psq",
                                      bufs=int(os.environ.get("KERNEL_PSQ", "1")))
                        pq = psq[:].rearrange("p (h w) -> p h w", w=W)
                        onv = ones[:].rearrange("p (h w) -> p h w", w=W)
                        for sub in (0, 2):
                            rs = r0 + sub
                            pqs = pq[:, sub:sub + 2, :]
                            bottom = last and ck == 3 and sub == 2
                            if ebq_mode:
                                # 2D 4-tap pool of ebq; no bias matmul
                                nc.tensor.matmul(pqs, eye[:],
                                                 ebq[:, rs:rs + 2, 0:W],
                                                 start=True, stop=False)
                                nc.tensor.matmul(pqs, eye[:],
                                                 ebq[:, rs:rs + 2, 1:W + 1],
                                                 start=False, stop=False)
                                if bottom:
                                    # rows 14,15: the h+1 taps clamp to 15
                                    for pi, pr in enumerate((2, 3)):
                                        st = pi == 1
                                        nc.tensor.matmul(
                                            pq[:, pr:pr + 1, :], eye[:],
                                            ebq[:, 15:16, 0:W],
                                            start=False, stop=False)
                                        nc.tensor.matmul(
                                            pq[:, pr:pr + 1, :], eye[:],
                                            ebq[:, 15:16, 1:W + 1],
                                            start=False, stop=st)
                                else:
                                    nc.tensor.matmul(pqs, eye[:],
                                                     ebq[:, rs + 1:rs + 3,
                                                         0:W],
                                                     start=False, stop=False)
                                    nc.tensor.matmul(pqs, eye[:],
                                                     ebq[:, rs + 1:rs + 3,
                                                         1:W + 1],
                                                     start=False, stop=True)
                                continue
                            if p1pe:
                                nc.tensor.matmul(pqs, eye[:],
                                                 ebx[:, rs:rs + 2, 0:W],
                                                 start=True, stop=False)
                                nc.tensor.matmul(pqs, eye[:],
                                                 ebx[:, rs:rs + 2, 1:W + 1],
                                                 start=False, stop=False)
                                if bottom:
                                    # rows 14,15: the h+1 taps clamp to 15
                                    for pr in (2, 3):
                                        nc.tensor.matmul(
                                            pq[:, pr:pr + 1, :], eye[:],
                                            ebx[:, 15:16, 0:W],
                                            start=False, stop=False)
                                        nc.tensor.matmul(
                                            pq[:, pr:pr + 1, :], eye[:],
                                            ebx[:, 15:16, 1:W + 1],
                                            start=False, stop=False)
                                else:
                                    nc.tensor.matmul(pqs, eye[:],
                                                     ebx[:, rs + 1:rs + 3, 0:W],
                                                     start=False, stop=False)
                                    nc.tensor.matmul(pqs, eye[:],
                                                     ebx[:, rs + 1:rs + 3,
                                                         1:W + 1],
                                                     start=False, stop=False)
                            else:
                                nc.tensor.matmul(pqs, eye[:],
                                                 p1[:, rs:rs + 2, :],
                                                 start=True, stop=False)
                                if bottom:
                                    # bottom image row: pv[15]=2*p1[15]+al8
                                    nc.tensor.matmul(pq[:, 2:3, :], eye[:],
                                                     p1[:, rs + 1:rs + 2, :],
                                                     start=False, stop=False)
                                    nc.tensor.matmul(pq[:, 3:4, :], eye[:],
                                                     p1[:, rs + 1:rs + 2, :],
                                                     start=False, stop=False)
                                else:
                                    nc.tensor.matmul(pqs, eye[:],
                                                     p1[:, rs + 1:rs + 3, :],
                                                     start=False, stop=False)
                            nc.tensor.matmul(pqs, dg_al8[:], onv,
                                             start=False, stop=True)
                        rec = pool.tile([P, 4 * W], f32, tag="rec",
                                        bufs=int(os.environ.get("KERNEL_REC", "4")))
                        nc.vector.reciprocal_approx_fast(rec[:], psq[:])
                        fn_c = fnum[:, r0:r0 + 4, :].rearrange(
                            "p h (w par) -> p h w par", par=2)
                        rc_c = rec[:].rearrange(
                            "p (h w par) -> p h w par", par=2, w=WO)
                        nc.gpsimd.tensor_tensor(
                            eq7v[:, r0:r0 + 4, :, :], fn_c, rc_c, Alu.mult)
                else:
                    pv = pool.tile([P, HT, W], f16, tag="pv")
                    if last:
                        nc.vector.tensor_tensor(
                            pv[:, 0:15, :], p1[:, 0:15, :], p1[:, 1:16, :],
                            Alu.add)
                        nc.vector.tensor_scalar_mul(
                            pv[:, 15:16, :], p1[:, 15:16, :], 2.0)
                    else:
                        nc.vector.tensor_tensor(pv[:], p1[:, 0:16, :],
                                                p1[:, 1:17, :], Alu.add)
                    den = pool.tile([P, HT, W], f32, tag="den")
                    nc.vector.tensor_scalar_add(den[:], pv[:], al8)
                    rec = pool.tile([P, HT, W], f32, tag="recf")
                    nc.vector.reciprocal_approx_fast(
                        rec[:].rearrange("p h w -> p (h w)"),
                        den[:].rearrange("p h w -> p (h w)"))
                    fn_c = fnum[:].rearrange("p h (w par) -> p h w par", par=2)
                    rc_c = rec[:].rearrange("p h (w par) -> p h w par", par=2)
                    nc.gpsimd.tensor_tensor(eq7v, fn_c, rc_c, Alu.mult)

                # ---- final pool: packed adds on the parity halves ----
                e0 = eq7[:, 0:HT * WO].rearrange("p (h w) -> p h w", w=WO)
                e1 = eq7[:, HT * WO:2 * HT * WO].rearrange(
                    "p (h w) -> p h w", w=WO)
                if pe_pool and int(os.environ.get("KERNEL_QPE", "0")):
                    o_t = pool.tile([P, HT // 2, WO], f32, tag="o", bufs=2)
                    # PE: o = e0[2j] + e0[2j+1] + e1[2j] + e1[2j+1] via 4
                    # accumulating taps (2 512-elem chunks), DVE evacuates.
                    po = pp.tile([P, (HT // 2) * WO], f32, tag="po",
                                 bufs=int(os.environ.get("KERNEL_PO", "2")))
                    pov = po[:].rearrange("p (h w) -> p h w", w=WO)
                    for jh in (0, 1):
                        pc = pov[:, 4 * jh:4 * jh + 4, :]
                        r0 = 8 * jh
                        nc.tensor.matmul(pc, eye[:], e0[:, r0:r0 + 8:2, :],
                                         start=True, stop=False)
                        nc.tensor.matmul(pc, eye[:], e0[:, r0 + 1:r0 + 8:2, :],
                                         start=False, stop=False)
                        nc.tensor.matmul(pc, eye[:], e1[:, r0:r0 + 8:2, :],
                                         start=False, stop=False)
                        nc.tensor.matmul(pc, eye[:], e1[:, r0 + 1:r0 + 8:2, :],
                                         start=False, stop=True)
                    nc.vector.tensor_copy(
                        o_t[:].rearrange("p h w -> p (h w)"), po[:])
                elif int(os.environ.get("KERNEL_O16", "1")):
                    q = pool.tile([P, HT, WO], f16, tag="q", bufs=2)
                    nc.vector.tensor_tensor(q[:], e0, e1, Alu.add)
                    o16 = pool.tile([P, HT // 2, WO], f16, tag="o16", bufs=2)
                    nc.vector.tensor_tensor(o16[:], q[:, 0:HT:2, :],
                                            q[:, 1:HT:2, :], Alu.add)
                    # gpsimd DMA casts f16 -> f32 on store
                    nc.gpsimd.dma_start(
                        od[:, (HT // 2) * i:(HT // 2) * (i + 1), :], o16[:])
                    return
                else:
                    q = pool.tile([P, HT, WO], f16, tag="q", bufs=2)
                    nc.vector.tensor_tensor(q[:], e0, e1, Alu.add)
                    o_t = pool.tile([P, HT // 2, WO], f32, tag="o", bufs=2)
                    nc.vector.tensor_tensor(o_t[:], q[:, 0:HT:2, :],
                                            q[:, 1:HT:2, :], Alu.add)
                nc.sync.dma_start(od[:, (HT // 2) * i:(HT // 2) * (i + 1), :],
                                  o_t[:])

            # software pipeline: front(i+1) is issued before back(i) so the
            # scheduler can overlap the PE/ACT head of one tile with the
            # DVE/GpSimd tail of the previous.
            LAG = int(os.environ.get("KERNEL_LAG", "1"))
            n_flat = rep * N_TILES
            live = {}
            for ii in range(n_flat + LAG):
                if ii < n_flat:
                    live[ii] = front(ii % N_TILES)
                jj = ii - LAG
                if jj >= 0:
                    xb_j, ebx_j = live.pop(jj)
                    back(jj % N_TILES, xb_j, ebx_j)
    nc.compile()
    _strip_act_table_loads(nc)
    return nc


def _get_nc(lam_val, alpha_val):
    key = ("nc", float(lam_val), float(alpha_val), STAGE)
    if key not in _cache:
        _cache[key] = _build(lam_val, alpha_val)
    return _cache[key]


def kernel(x, lam, alpha):
    if not int(os.environ.get("KERNEL_TRACE", "0")):
        os.environ["BASS_NEVER_TRACE"] = "1"
    # The harness may pin JAX_PLATFORMS=cpu for its jax reference; that would
    # mask the axon NeuronCore devices this kernel dispatches to. Clear it
    # before jax's backend initializes (no-op if jax already initialized).
    jp = os.environ.get("JAX_PLATFORMS")
    if jp and "axon" not in jp:
        del os.environ["JAX_PLATFORMS"]
    import concourse.bass_utils as bass_utils

    x = np.ascontiguousarray(np.asarray(x, dtype=np.float32))
    lam = np.asarray(lam, dtype=np.float32).reshape(1, 1)
    alpha = np.asarray(alpha, dtype=np.float32).reshape(1, 1)
    assert x.shape == (B, C, H, W)

    nc = _get_nc(float(lam[0, 0]), float(alpha[0, 0]))
    in_maps = []
    for i in range(N_CORES):
        shard = x[i * B_LOC:(i + 1) * B_LOC].reshape(P, H * W)
        in_maps.append({"x": np.ascontiguousarray(shard),
                        "lam": lam, "alpha": alpha})

    res = bass_utils.run_bass_kernel_spmd(
        nc, in_maps, core_ids=list(range(N_CORES)),
        trace=bool(int(os.environ.get("KERNEL_TRACE", "0"))))
    _cache["last_results"] = res

    out = np.empty((B, C, HO, WO), dtype=np.float32)
    for i in range(N_CORES):
        out[i * B_LOC:(i + 1) * B_LOC] = \
            res.results[i]["out"].reshape(B_LOC, C, HO, WO)
    return out
